# revision 2
# baseline (speedup 1.0000x reference)
"""Causal dilated 1D conv (KW=4, dilation=8) as shifted matmuls on 8 TRN2 cores.

out[b,o,t] = sum_{k,c} W[o, c*4+k] * x[b, c, t + k*8 - 24]

Sharding: data-parallel over batch (16 batches -> 2 per core). Each core runs
an identical program: weights stationary in SBUF, x streamed in 512-wide time
blocks (+24 halo), 16 accumulating matmuls (4 c-chunks x 4 taps) per
(out-chunk, time-block) PSUM group. Matmuls run in float32r (fp32 data,
FP22 multiply) which streams at 1 cycle/row for free-dim >= 256.
"""

import numpy as np

B = 16
C_IN = 512
C_OUT = 512
T = 8192
KW = 4
DIL = 8
PAD = (KW - 1) * DIL  # 24

N_CORES = 8
B_PER = B // N_CORES  # 2
P = 128
TBLK = 512
NT = T // TBLK        # 16
NCC = C_IN // P       # 4
NOC = C_OUT // P      # 4

_cache = {}


def _build():
    import concourse.tile as tile
    from concourse import bacc, mybir

    nc = bacc.Bacc("TRN2", target_bir_lowering=False, debug=False,
                   num_devices=N_CORES)
    x = nc.dram_tensor("x", [B_PER, C_IN, T + PAD], mybir.dt.float32r,
                       kind="ExternalInput").ap()
    wt = nc.dram_tensor("wt", [KW, C_IN, C_OUT], mybir.dt.float32r,
                        kind="ExternalInput").ap()
    out = nc.dram_tensor("out", [B_PER, C_OUT, T], mybir.dt.float32,
                         kind="ExternalOutput").ap()
    f32 = mybir.dt.float32
    f32r = mybir.dt.float32r

    with tile.TileContext(nc) as tc:
        with tc.tile_pool(name="wpool", bufs=1) as wpool, \
             tc.tile_pool(name="xpool", bufs=3) as xpool, \
             tc.tile_pool(name="opool", bufs=4) as opool, \
             tc.tile_pool(name="pspool", bufs=8, space="PSUM") as pspool:

            # Weights resident for the whole kernel: [c=128, o=512] per (tap, c-chunk).
            wtiles = []
            for k in range(KW):
                row = []
                for cc in range(NCC):
                    wtile = wpool.tile([P, C_OUT], f32r, name=f"w_{k}_{cc}",
                                       tag=f"w_{k}_{cc}")
                    nc.sync.dma_start(wtile[:], wt[k, cc * P:(cc + 1) * P, :])
                    row.append(wtile)
                wtiles.append(row)

            for b in range(B_PER):
                for tb in range(NT):
                    xts = []
                    for cc in range(NCC):
                        xt = xpool.tile([P, TBLK + PAD], f32r,
                                        name=f"xt{cc}", tag=f"xt{cc}")
                        nc.sync.dma_start(
                            xt[:],
                            x[b, cc * P:(cc + 1) * P,
                              tb * TBLK: tb * TBLK + TBLK + PAD])
                        xts.append(xt)
                    for oc in range(NOC):
                        ps = pspool.tile([P, TBLK], f32, name="ps", tag="ps")
                        idx = 0
                        for cc in range(NCC):
                            for k in range(KW):
                                nc.tensor.matmul(
                                    ps[:],
                                    wtiles[k][cc][:, oc * P:(oc + 1) * P],
                                    xts[cc][:, k * DIL: k * DIL + TBLK],
                                    start=(idx == 0),
                                    stop=(idx == NCC * KW - 1),
                                )
                                idx += 1
                        ot = opool.tile([P, TBLK], f32, name="ot", tag="ot")
                        nc.vector.tensor_copy(ot[:], ps[:])
                        nc.sync.dma_start(
                            out[b, oc * P:(oc + 1) * P,
                                tb * TBLK:(tb + 1) * TBLK],
                            ot[:])

    nc.compile()
    return nc


def _get_nc():
    if "nc" not in _cache:
        _cache["nc"] = _build()
    return _cache["nc"]


def _make_in_maps(x, W):
    xpad = np.pad(np.ascontiguousarray(x, dtype=np.float32),
                  ((0, 0), (0, 0), (PAD, 0)))
    w = np.ascontiguousarray(W, dtype=np.float32).reshape(C_OUT, C_IN, KW)
    wt = np.ascontiguousarray(np.transpose(w, (2, 1, 0)))  # [KW, C_IN, C_OUT]
    return [{"x": np.ascontiguousarray(xpad[i * B_PER:(i + 1) * B_PER]),
             "wt": wt} for i in range(N_CORES)]


def kernel(x, W):
    from concourse.bass_utils import run_bass_kernel_spmd

    nc = _get_nc()
    in_maps = _make_in_maps(x, W)
    res = run_bass_kernel_spmd(nc, in_maps, list(range(N_CORES)))
    return np.concatenate([r["out"] for r in res.results], axis=0)
